# revision 5
# baseline (speedup 1.0000x reference)
"""Trainium2 Bass kernel for nn_L2PppMaskAttn (topk_masking).

Math reformulation of the reference:
  - a_k = sum(l2norm(K[idx]) * l2norm(A[idx])) depends only on (layer, prompt):
    precompute s[l,p] = <K,A> / (||K|| ||A||) once per layer on-device.
  - top-5 ranking over prompts is invariant to q normalization (positive
    per-row scale), so scores u[b,p] = <x[b,l], K[l,p]> / ||K[l,p]|| suffice.
  - out[l,b] = sum_{p in top5} s[l,p] * P[l,p] = (mask_row .* s) @ P_flat[l],
    a dense [B,100] @ [100, 6144] matmul per layer (topk -> masking).

v5 notes (356us baseline -> targeting the ~222us memory roofline):
  - Host passes x, K, A pre-transposed (d on partitions).
  - ||K||^2/||A||^2/<K,A>: square/mult partials, strided 3-op DVE chunk
    fold, then only 3 ones-moving matmuls into a [100,4] column block
    (fp32-stationary matmuls double-emit HI/LO, so keep PE count low).
  - Norm chain entirely in [100,k] column form (single-partition DVE ops
    are ~7x slower; 1-lane only).
  - Scores computed TRANSPOSED so the 1/||K|| scale is a native
    per-partition tensor_scalar; one PE transpose back for the top-k.
  - top-5 threshold via the DVE InstMax top-8 instruction (1 op).
  - DMA triggers must not block compute engines: ALL loads issue from
    nc.sync (pure DMA engine; slot waits harmless), stores from
    nc.scalar (ACT reaches the trigger right after producing ob, so the
    wait is ~0). GpSimd/SWDGE unused.
  - Output matmul pairs share a [128,1024] 2-bank PSUM tile -> 6 wider
    PSUM->SBUF copies per layer instead of 12.

Sharding: data-parallel over batch, 8 cores x 128 rows; K/A/P replicated.
"""

import sys

sys.path.insert(0, "/opt/trn_rl_repo")

import numpy as np

B, L, P_N, LP, D = 1024, 12, 100, 8, 768
N_CORES = 8
BS = B // N_CORES  # 128 batch rows per core
NF = LP * D  # 6144 flattened output features per layer
NC6 = D // 128  # 6 contraction chunks of 128
TOP_K = 5

_CACHE = {}


def _build_nc():
    if "nc" in _CACHE:
        return _CACHE["nc"]

    from contextlib import ExitStack

    import concourse.bass as bass
    import concourse.bacc as bacc
    import concourse.mybir as mybir
    from concourse import masks
    from concourse.tile import TileContext

    f32 = mybir.dt.float32
    f32r = mybir.dt.float32r
    AX = mybir.AxisListType
    OP = mybir.AluOpType
    AF = mybir.ActivationFunctionType

    nc = bacc.Bacc(
        "TRN2",
        target_bir_lowering=False,
        debug=False,
        num_devices=N_CORES,
    )

    # Host-side layouts (see _pack_inputs):
    #  xt[l, p, c*128 + b]        = x_core[b, l, c*128 + p]
    #  kat[l, p, c*100 + j]       = K[l, j, c*128 + p]      (cols 0..599)
    #  kat[l, p, 600 + c*100 + j] = A[l, j, c*128 + p]      (cols 600..1199)
    #  p[l, j, :]                 = P[l, j].reshape(NF)
    xt_d = nc.declare_dram_parameter("xt", [L, BS, D], f32, isOutput=False)
    kat_d = nc.declare_dram_parameter(
        "kat", [L, 128, 2 * NC6 * P_N], f32, isOutput=False
    )
    p_d = nc.declare_dram_parameter("p", [L, P_N, NF], f32r, isOutput=False)
    o_d = nc.declare_dram_parameter("o", [L, BS, NF], f32, isOutput=True)

    with TileContext(nc) as tc, ExitStack() as ctx:
        pool = lambda name, bufs, **kw: ctx.enter_context(
            tc.tile_pool(name=name, bufs=bufs, **kw)
        )
        const = pool("const", 1)
        katp = pool("katp", 3)
        xtp = pool("xtp", 3)
        ppool = pool("pp", 4)
        sqp = pool("sqp", 2)
        foldp = pool("foldp", 2)
        smp = pool("smp", 3)
        obp = pool("obp", 4)
        ps_sp = pool("ps_sp", 3, space="PSUM")  # red/scT/St/mt share one tag
        ps_o = pool("ps_o", 2, space="PSUM")    # [128,1024] = 2 banks each

        ident = const.tile([128, 128], f32)
        masks.make_identity(nc, ident[:])
        ones_col = const.tile([128, 1], f32)
        nc.vector.memset(ones_col[:], 1.0)

        for l in range(L):
            # ---- all loads issue from the SP ring (nc.sync) ----
            kat = katp.tile([128, 2 * NC6 * P_N], f32)
            nc.sync.dma_start(kat[:], kat_d[l])
            xt = xtp.tile([BS, D], f32)
            nc.sync.dma_start(xt[:], xt_d[l])
            p_sb = ppool.tile([P_N, NF], f32r)
            nc.sync.dma_start(p_sb[:], p_d[l])

            # ---- partial products: K^2 | A^2 | K*A  (d on partitions) ----
            sq = sqp.tile([128, 1800], f32)
            nc.scalar.activation(sq[:, 0:600], kat[:, 0:600], AF.Square)
            nc.scalar.activation(sq[:, 600:1200], kat[:, 600:1200], AF.Square)
            nc.vector.tensor_tensor(
                sq[:, 1200:1800], kat[:, 0:600], kat[:, 600:1200], op=OP.mult
            )

            # ---- fold the 6 d-chunks of each quantity: [128,1800]->[128,300]
            tq = foldp.tile([128, 900], f32, tag="tq")
            f300 = foldp.tile([128, 300], f32, tag="f300")
            sq_v = sq[:].rearrange("p (q c j) -> p q c j", q=3, c=6)
            tq_v = tq[:].rearrange("p (q c j) -> p q c j", q=3, c=3)
            f_v = f300[:].rearrange("p (q j) -> p q j", q=3)
            nc.vector.tensor_tensor(tq_v, sq_v[:, :, 0:3], sq_v[:, :, 3:6], op=OP.add)
            nc.vector.tensor_tensor(f_v, tq_v[:, :, 0], tq_v[:, :, 1], op=OP.add)
            nc.vector.tensor_tensor(f_v, f_v, tq_v[:, :, 2], op=OP.add)

            # ---- partition-dim reduce: 3 matmuls into [100,4] columns ----
            red = ps_sp.tile([P_N, 4], f32, tag="sp")
            for q in range(3):
                nc.tensor.matmul(
                    red[:, q : q + 1],
                    f300[:, q * 100 : (q + 1) * 100],
                    ones_col[:],
                    start=True,
                    stop=True,
                )
            sqs = smp.tile([P_N, 3], f32, tag="sqs")
            nc.scalar.copy(sqs[:], red[:, 0:3])

            # ---- rsqrt of ||K||^2, ||A||^2 with one Newton step ----
            srt = smp.tile([P_N, 2], f32, tag="srt")
            nc.scalar.activation(srt[:], sqs[:, 0:2], AF.Sqrt)
            y0 = smp.tile([P_N, 2], f32, tag="y0")
            nc.vector.reciprocal(y0[:], srt[:])
            t1 = smp.tile([P_N, 2], f32, tag="t1")
            nc.vector.tensor_tensor(t1[:], y0[:], y0[:], op=OP.mult)
            nc.vector.tensor_tensor(t1[:], t1[:], sqs[:, 0:2], op=OP.mult)
            nc.vector.tensor_scalar(t1[:], t1[:], -0.5, 1.5, OP.mult, OP.add)
            rs2 = smp.tile([P_N, 2], f32, tag="rs2")
            nc.vector.tensor_tensor(rs2[:], t1[:], y0[:], op=OP.mult)

            # s[p] = <K,A> * rsK * rsA   (column form, for the wt scale)
            s_col = smp.tile([P_N, 1], f32, tag="scol")
            nc.vector.tensor_tensor(s_col[:], rs2[:, 0:1], rs2[:, 1:2], op=OP.mult)
            nc.vector.tensor_tensor(s_col[:], s_col[:], sqs[:, 2:3], op=OP.mult)

            # ---- scores transposed: [100p, 128b] = sum_c K_c.T @ x_c ----
            scT = ps_sp.tile([P_N, BS], f32, tag="sp")
            for c in range(NC6):
                nc.tensor.matmul(
                    scT[:],
                    kat[:, c * P_N : (c + 1) * P_N],
                    xt[:, c * 128 : (c + 1) * 128],
                    start=(c == 0),
                    stop=(c == NC6 - 1),
                )
            su = smp.tile([P_N, BS], f32, tag="su")
            nc.vector.tensor_scalar_mul(su[:], scT[:], rs2[:, 0:1])

            # back to [128b, 100p] for the row top-k
            St = ps_sp.tile([BS, P_N], f32, tag="sp")
            nc.tensor.transpose(St[:], su[:], ident[:P_N, :P_N])
            Ssb = smp.tile([BS, P_N], f32, tag="Ssb")
            nc.scalar.copy(Ssb[:], St[:])

            # ---- top-8 per row in one DVE op; threshold = 5th largest ----
            m8 = smp.tile([BS, 8], f32, tag="m8")
            nc.vector.max(m8[:], Ssb[:])
            mask = smp.tile([BS, P_N], f32, tag="mask")
            nc.vector.tensor_scalar(
                mask[:], Ssb[:], m8[:, TOP_K - 1 : TOP_K], None, OP.is_ge
            )

            # W^T = mask^T * s -> [100, 128]
            mt = ps_sp.tile([P_N, BS], f32, tag="sp")
            nc.tensor.transpose(mt[:], mask[:], ident[:])
            wt = smp.tile([P_N, BS], f32r, tag="wt")
            nc.vector.tensor_scalar_mul(wt[:], mt[:], s_col[:])

            # ---- out[l] = W @ P_flat : paired-bank PSUM, 2 half stores ----
            for h in range(2):
                ob = obp.tile([BS, NF // 2], f32)
                for j in range(3):
                    po = ps_o.tile([BS, 1024], f32)
                    for g in range(2):
                        n = h * 6 + j * 2 + g
                        nc.tensor.matmul(
                            po[:, g * 512 : (g + 1) * 512],
                            wt[:],
                            p_sb[:, n * 512 : (n + 1) * 512],
                            start=True,
                            stop=True,
                        )
                    if j == 2:
                        nc.scalar.copy(ob[:, j * 1024 : (j + 1) * 1024], po[:])
                    else:
                        nc.vector.tensor_copy(
                            ob[:, j * 1024 : (j + 1) * 1024], po[:]
                        )
                nc.scalar.dma_start(
                    o_d[l][:, h * (NF // 2) : (h + 1) * (NF // 2)], ob[:]
                )

    nc.compile()
    _CACHE["nc"] = nc
    return nc


def _pack_inputs(x_query, K_all, A_all, P_all):
    x = np.ascontiguousarray(np.asarray(x_query, dtype=np.float32))
    k = np.asarray(K_all, dtype=np.float32)
    a = np.asarray(A_all, dtype=np.float32)
    p = np.ascontiguousarray(
        np.asarray(P_all, dtype=np.float32).reshape(L, P_N, NF)
    )

    def t_pool(m):  # [L,P,D] -> [L,128,6*P]: out[l,p,c*P+j] = m[l,j,c*128+p]
        r = m.transpose(0, 2, 1).reshape(L, NC6, 128, P_N)
        return r.transpose(0, 2, 1, 3).reshape(L, 128, NC6 * P_N)

    kat = np.ascontiguousarray(np.concatenate([t_pool(k), t_pool(a)], axis=2))

    xts = []
    for c in range(N_CORES):
        xc = x[c * BS : (c + 1) * BS]  # [128, L, D]
        # xt[l,p,c6*128+b] = xc[b,l,c6*128+p]
        r = xc.transpose(1, 2, 0).reshape(L, NC6, 128, BS)
        xts.append(
            np.ascontiguousarray(r.transpose(0, 2, 1, 3).reshape(L, BS, D))
        )
    return xts, kat, p


def _run(x_query, K_all, A_all, P_all, trace=False, tmpdir=None):
    from concourse.bass_utils import run_bass_kernel_spmd

    xts, kat, p = _pack_inputs(x_query, K_all, A_all, P_all)

    nc = _build_nc()
    in_maps = [{"xt": xts[c], "kat": kat, "p": p} for c in range(N_CORES)]
    br = run_bass_kernel_spmd(
        nc, in_maps, list(range(N_CORES)), trace=trace, tmpdir=tmpdir
    )
    out = np.stack([r["o"] for r in br.results], axis=0)  # [8, L, BS, NF]
    out = out.transpose(1, 0, 2, 3).reshape(L, B, LP, D)
    return out, br


def kernel(x_query, K_all, A_all, P_all):
    out, _ = _run(x_query, K_all, A_all, P_all)
    return out


# revision 7
# speedup vs baseline: 1.0435x; 1.0435x over previous
"""Trainium2 Bass kernel for nn_L2PppMaskAttn (topk_masking).

Math reformulation of the reference:
  - a_k = sum(l2norm(K[idx]) * l2norm(A[idx])) depends only on (layer, prompt):
    precompute s[l,p] = <K,A> / (||K|| ||A||) once per layer on-device.
  - top-5 ranking over prompts is invariant to q normalization (positive
    per-row scale), so scores u[b,p] = <x[b,l], K[l,p]> / ||K[l,p]|| suffice.
  - out[l,b] = sum_{p in top5} s[l,p] * P[l,p] = (mask_row .* s) @ P_flat[l],
    a dense [B,100] @ [100, 6144] matmul per layer (topk -> masking).

v5 notes (356us baseline -> targeting the ~222us memory roofline):
  - Host passes x, K, A pre-transposed (d on partitions).
  - ||K||^2/||A||^2/<K,A>: square/mult partials, strided 3-op DVE chunk
    fold, then only 3 ones-moving matmuls into a [100,4] column block
    (fp32-stationary matmuls double-emit HI/LO, so keep PE count low).
  - Norm chain entirely in [100,k] column form (single-partition DVE ops
    are ~7x slower; 1-lane only).
  - Scores computed TRANSPOSED so the 1/||K|| scale is a native
    per-partition tensor_scalar; one PE transpose back for the top-k.
  - top-5 threshold via the DVE InstMax top-8 instruction (1 op).
  - DMA triggers must not block compute engines: ALL loads issue from
    nc.sync (pure DMA engine; slot waits harmless), stores from
    nc.scalar (ACT reaches the trigger right after producing ob, so the
    wait is ~0). GpSimd/SWDGE unused.
  - Output matmul pairs share a [128,1024] 2-bank PSUM tile -> 6 wider
    PSUM->SBUF copies per layer instead of 12.

Sharding: data-parallel over batch, 8 cores x 128 rows; K/A/P replicated.
"""

import sys

sys.path.insert(0, "/opt/trn_rl_repo")

import numpy as np

B, L, P_N, LP, D = 1024, 12, 100, 8, 768
N_CORES = 8
BS = B // N_CORES  # 128 batch rows per core
NF = LP * D  # 6144 flattened output features per layer
NC6 = D // 128  # 6 contraction chunks of 128
TOP_K = 5

_CACHE = {}


def _build_nc():
    if "nc" in _CACHE:
        return _CACHE["nc"]

    from contextlib import ExitStack

    import concourse.bass as bass
    import concourse.bacc as bacc
    import concourse.mybir as mybir
    from concourse import masks
    from concourse.tile import TileContext

    f32 = mybir.dt.float32
    f32r = mybir.dt.float32r
    AX = mybir.AxisListType
    OP = mybir.AluOpType
    AF = mybir.ActivationFunctionType

    nc = bacc.Bacc(
        "TRN2",
        target_bir_lowering=False,
        debug=False,
        num_devices=N_CORES,
    )

    # Host-side layouts (see _pack_inputs):
    #  xt[l, p, c*128 + b]        = x_core[b, l, c*128 + p]
    #  kat[l, p, c*100 + j]       = K[l, j, c*128 + p]      (cols 0..599)
    #  kat[l, p, 600 + c*100 + j] = A[l, j, c*128 + p]      (cols 600..1199)
    #  p[l, j, :]                 = P[l, j].reshape(NF)
    xt_d = nc.declare_dram_parameter("xt", [L, BS, D], f32, isOutput=False)
    kat_d = nc.declare_dram_parameter(
        "kat", [L, 128, 2 * NC6 * P_N], f32, isOutput=False
    )
    p_d = nc.declare_dram_parameter("p", [L, P_N, NF], f32r, isOutput=False)
    o_d = nc.declare_dram_parameter("o", [L, BS, NF], f32, isOutput=True)

    with TileContext(nc) as tc, ExitStack() as ctx:
        pool = lambda name, bufs, **kw: ctx.enter_context(
            tc.tile_pool(name=name, bufs=bufs, **kw)
        )
        const = pool("const", 1)
        katp = pool("katp", 3)
        xtp = pool("xtp", 3)
        ppool = pool("pp", 4)
        sqp = pool("sqp", 2)
        foldp = pool("foldp", 2)
        smp = pool("smp", 3)
        obp = pool("obp", 4)
        ps_sp = pool("ps_sp", 3, space="PSUM")  # red/scT/St/mt share one tag
        ps_o = pool("ps_o", 2, space="PSUM")    # [128,1024] = 2 banks each

        ident = const.tile([128, 128], f32)
        masks.make_identity(nc, ident[:])
        ones_col = const.tile([128, 1], f32)
        nc.vector.memset(ones_col[:], 1.0)

        def out_stage(lp, wt, p_sb):
            # out[lp] = W @ P_flat: paired-bank PSUM, 2 half stores.
            # Inputs were finished during the previous iteration, so these
            # matmuls/copies fill every engine's dependency stalls while
            # the CURRENT layer's selection chain is still in flight.
            for h in range(2):
                ob = obp.tile([BS, NF // 2], f32)
                for j in range(3):
                    po = ps_o.tile([BS, 1024], f32)
                    for g in range(2):
                        n = h * 6 + j * 2 + g
                        nc.tensor.matmul(
                            po[:, g * 512 : (g + 1) * 512],
                            wt[:],
                            p_sb[:, n * 512 : (n + 1) * 512],
                            start=True,
                            stop=True,
                        )
                    if j == 2:
                        nc.scalar.copy(ob[:, j * 1024 : (j + 1) * 1024], po[:])
                    else:
                        nc.vector.tensor_copy(
                            ob[:, j * 1024 : (j + 1) * 1024], po[:]
                        )
                nc.scalar.dma_start(
                    o_d[lp][:, h * (NF // 2) : (h + 1) * (NF // 2)], ob[:]
                )

        prev = None
        for l in range(L):
            # ---- all loads issue from the SP ring (nc.sync) ----
            kat = katp.tile([128, 2 * NC6 * P_N], f32)
            nc.sync.dma_start(kat[:], kat_d[l])
            xt = xtp.tile([BS, D], f32)
            nc.sync.dma_start(xt[:], xt_d[l])
            p_sb = ppool.tile([P_N, NF], f32r)
            nc.sync.dma_start(p_sb[:], p_d[l])

            # ---- previous layer's output stage (software pipelining) ----
            if prev is not None:
                out_stage(l - 1, *prev)

            # ---- partial products: K^2 | A^2 | K*A  (d on partitions) ----
            sq = sqp.tile([128, 1800], f32)
            nc.scalar.activation(sq[:, 0:600], kat[:, 0:600], AF.Square)
            nc.scalar.activation(sq[:, 600:1200], kat[:, 600:1200], AF.Square)
            nc.vector.tensor_tensor(
                sq[:, 1200:1800], kat[:, 0:600], kat[:, 600:1200], op=OP.mult
            )

            # ---- fold the 6 d-chunks of each quantity: [128,1800]->[128,300]
            tq = foldp.tile([128, 900], f32, tag="tq")
            f300 = foldp.tile([128, 300], f32, tag="f300")
            sq_v = sq[:].rearrange("p (q c j) -> p q c j", q=3, c=6)
            tq_v = tq[:].rearrange("p (q c j) -> p q c j", q=3, c=3)
            f_v = f300[:].rearrange("p (q j) -> p q j", q=3)
            nc.vector.tensor_tensor(tq_v, sq_v[:, :, 0:3], sq_v[:, :, 3:6], op=OP.add)
            nc.vector.tensor_tensor(f_v, tq_v[:, :, 0], tq_v[:, :, 1], op=OP.add)
            nc.vector.tensor_tensor(f_v, f_v, tq_v[:, :, 2], op=OP.add)

            # ---- partition-dim reduce: 3 matmuls into [100,4] columns ----
            red = ps_sp.tile([P_N, 4], f32, tag="sp")
            for q in range(3):
                nc.tensor.matmul(
                    red[:, q : q + 1],
                    f300[:, q * 100 : (q + 1) * 100],
                    ones_col[:],
                    start=True,
                    stop=True,
                )
            sqs = smp.tile([P_N, 3], f32, tag="sqs")
            nc.scalar.copy(sqs[:], red[:, 0:3])

            # ---- rsqrt of ||K||^2, ||A||^2 with one Newton step ----
            srt = smp.tile([P_N, 2], f32, tag="srt")
            nc.scalar.activation(srt[:], sqs[:, 0:2], AF.Sqrt)
            y0 = smp.tile([P_N, 2], f32, tag="y0")
            nc.vector.reciprocal(y0[:], srt[:])
            t1 = smp.tile([P_N, 2], f32, tag="t1")
            nc.vector.tensor_tensor(t1[:], y0[:], y0[:], op=OP.mult)
            nc.vector.tensor_tensor(t1[:], t1[:], sqs[:, 0:2], op=OP.mult)
            nc.vector.tensor_scalar(t1[:], t1[:], -0.5, 1.5, OP.mult, OP.add)
            rs2 = smp.tile([P_N, 2], f32, tag="rs2")
            nc.vector.tensor_tensor(rs2[:], t1[:], y0[:], op=OP.mult)

            # s[p] = <K,A> * rsK * rsA   (column form, for the wt scale)
            s_col = smp.tile([P_N, 1], f32, tag="scol")
            nc.vector.tensor_tensor(s_col[:], rs2[:, 0:1], rs2[:, 1:2], op=OP.mult)
            nc.vector.tensor_tensor(s_col[:], s_col[:], sqs[:, 2:3], op=OP.mult)

            # ---- scores transposed: [100p, 128b] = sum_c K_c.T @ x_c ----
            scT = ps_sp.tile([P_N, BS], f32, tag="sp")
            for c in range(NC6):
                nc.tensor.matmul(
                    scT[:],
                    kat[:, c * P_N : (c + 1) * P_N],
                    xt[:, c * 128 : (c + 1) * 128],
                    start=(c == 0),
                    stop=(c == NC6 - 1),
                )
            su = smp.tile([P_N, BS], f32, tag="su")
            nc.vector.tensor_scalar_mul(su[:], scT[:], rs2[:, 0:1])

            # back to [128b, 100p] for the row top-k
            St = ps_sp.tile([BS, P_N], f32, tag="sp")
            nc.tensor.transpose(St[:], su[:], ident[:P_N, :P_N])
            Ssb = smp.tile([BS, P_N], f32, tag="Ssb")
            nc.scalar.copy(Ssb[:], St[:])

            # ---- top-8 per row in one DVE op; threshold = 5th largest ----
            m8 = smp.tile([BS, 8], f32, tag="m8")
            nc.vector.max(m8[:], Ssb[:])
            mask = smp.tile([BS, P_N], f32, tag="mask")
            nc.vector.tensor_scalar(
                mask[:], Ssb[:], m8[:, TOP_K - 1 : TOP_K], None, OP.is_ge
            )

            # W^T = mask^T * s -> [100, 128]
            mt = ps_sp.tile([P_N, BS], f32, tag="sp")
            nc.tensor.transpose(mt[:], mask[:], ident[:])
            wt = smp.tile([P_N, BS], f32r, tag="wt")
            nc.vector.tensor_scalar_mul(wt[:], mt[:], s_col[:])

            prev = (wt, p_sb)

        out_stage(L - 1, *prev)

    nc.compile()
    _CACHE["nc"] = nc
    return nc


def _pack_inputs(x_query, K_all, A_all, P_all):
    x = np.ascontiguousarray(np.asarray(x_query, dtype=np.float32))
    k = np.asarray(K_all, dtype=np.float32)
    a = np.asarray(A_all, dtype=np.float32)
    p = np.ascontiguousarray(
        np.asarray(P_all, dtype=np.float32).reshape(L, P_N, NF)
    )

    def t_pool(m):  # [L,P,D] -> [L,128,6*P]: out[l,p,c*P+j] = m[l,j,c*128+p]
        r = m.transpose(0, 2, 1).reshape(L, NC6, 128, P_N)
        return r.transpose(0, 2, 1, 3).reshape(L, 128, NC6 * P_N)

    kat = np.ascontiguousarray(np.concatenate([t_pool(k), t_pool(a)], axis=2))

    xts = []
    for c in range(N_CORES):
        xc = x[c * BS : (c + 1) * BS]  # [128, L, D]
        # xt[l,p,c6*128+b] = xc[b,l,c6*128+p]
        r = xc.transpose(1, 2, 0).reshape(L, NC6, 128, BS)
        xts.append(
            np.ascontiguousarray(r.transpose(0, 2, 1, 3).reshape(L, BS, D))
        )
    return xts, kat, p


def _run(x_query, K_all, A_all, P_all, trace=False, tmpdir=None):
    from concourse.bass_utils import run_bass_kernel_spmd

    xts, kat, p = _pack_inputs(x_query, K_all, A_all, P_all)

    nc = _build_nc()
    in_maps = [{"xt": xts[c], "kat": kat, "p": p} for c in range(N_CORES)]
    br = run_bass_kernel_spmd(
        nc, in_maps, list(range(N_CORES)), trace=trace, tmpdir=tmpdir
    )
    out = np.stack([r["o"] for r in br.results], axis=0)  # [8, L, BS, NF]
    out = out.transpose(1, 0, 2, 3).reshape(L, B, LP, D)
    return out, br


def kernel(x_query, K_all, A_all, P_all):
    out, _ = _run(x_query, K_all, A_all, P_all)
    return out


# revision 8
# speedup vs baseline: 1.1051x; 1.0590x over previous
"""Trainium2 Bass kernel for nn_L2PppMaskAttn (topk_masking).

Math reformulation of the reference:
  - a_k = sum(l2norm(K[idx]) * l2norm(A[idx])) depends only on (layer, prompt):
    precompute s[l,p] = <K,A> / (||K|| ||A||) once per layer on-device.
  - top-5 ranking over prompts is invariant to q normalization (positive
    per-row scale), so scores u[b,p] = <x[b,l], K[l,p]> / ||K[l,p]|| suffice.
  - out[l,b] = sum_{p in top5} s[l,p] * P[l,p] = (mask_row .* s) @ P_flat[l],
    a dense [B,100] @ [100, 6144] matmul per layer (topk -> masking).

v5 notes (356us baseline -> targeting the ~222us memory roofline):
  - Host passes x, K, A pre-transposed (d on partitions).
  - ||K||^2/||A||^2/<K,A>: square/mult partials, strided 3-op DVE chunk
    fold, then only 3 ones-moving matmuls into a [100,4] column block
    (fp32-stationary matmuls double-emit HI/LO, so keep PE count low).
  - Norm chain entirely in [100,k] column form (single-partition DVE ops
    are ~7x slower; 1-lane only).
  - Scores computed TRANSPOSED so the 1/||K|| scale is a native
    per-partition tensor_scalar; one PE transpose back for the top-k.
  - top-5 threshold via the DVE InstMax top-8 instruction (1 op).
  - DMA triggers must not block compute engines: ALL loads issue from
    nc.sync (pure DMA engine; slot waits harmless), stores from
    nc.scalar (ACT reaches the trigger right after producing ob, so the
    wait is ~0). GpSimd/SWDGE unused.
  - Output matmul pairs share a [128,1024] 2-bank PSUM tile -> 6 wider
    PSUM->SBUF copies per layer instead of 12.

Sharding: data-parallel over batch, 8 cores x 128 rows; K/A/P replicated.
"""

import sys

sys.path.insert(0, "/opt/trn_rl_repo")

import numpy as np

B, L, P_N, LP, D = 1024, 12, 100, 8, 768
N_CORES = 8
BS = B // N_CORES  # 128 batch rows per core
NF = LP * D  # 6144 flattened output features per layer
NC6 = D // 128  # 6 contraction chunks of 128
TOP_K = 5

_CACHE = {}


def _build_nc():
    if "nc" in _CACHE:
        return _CACHE["nc"]

    from contextlib import ExitStack

    import concourse.bass as bass
    import concourse.bacc as bacc
    import concourse.mybir as mybir
    from concourse import masks
    from concourse.tile import TileContext

    f32 = mybir.dt.float32
    f32r = mybir.dt.float32r
    AX = mybir.AxisListType
    OP = mybir.AluOpType
    AF = mybir.ActivationFunctionType

    nc = bacc.Bacc(
        "TRN2",
        target_bir_lowering=False,
        debug=False,
        num_devices=N_CORES,
    )

    # Host-side layouts (see _pack_inputs):
    #  xt[l, p, c*128 + b]        = x_core[b, l, c*128 + p]
    #  kat[l, p, c*100 + j]       = K[l, j, c*128 + p]      (cols 0..599)
    #  kat[l, p, 600 + c*100 + j] = A[l, j, c*128 + p]      (cols 600..1199)
    #  p[l, j, :]                 = P[l, j].reshape(NF)
    xt_d = nc.declare_dram_parameter("xt", [L, BS, D], f32, isOutput=False)
    kat_d = nc.declare_dram_parameter(
        "kat", [L, 128, 2 * NC6 * P_N], f32, isOutput=False
    )
    p_d = nc.declare_dram_parameter("p", [L, 2, P_N, NF // 2], f32r, isOutput=False)
    o_d = nc.declare_dram_parameter("o", [L, BS, NF], f32, isOutput=True)

    with TileContext(nc) as tc, ExitStack() as ctx:
        pool = lambda name, bufs, **kw: ctx.enter_context(
            tc.tile_pool(name=name, bufs=bufs, **kw)
        )
        const = pool("const", 1)
        katp = pool("katp", 6)
        xtp = pool("xtp", 6)
        ppool = pool("pp", 6)
        sqp = pool("sqp", 2)
        foldp = pool("foldp", 2)
        smp = pool("smp", 3)
        obp = pool("obp", 4)
        ps_sp = pool("ps_sp", 3, space="PSUM")  # red/scT/St/mt share one tag
        ps_o = pool("ps_o", 2, space="PSUM")    # [128,1024] = 2 banks each

        ident = const.tile([128, 128], f32)
        masks.make_identity(nc, ident[:])
        ones_col = const.tile([128, 1], f32)
        nc.vector.memset(ones_col[:], 1.0)

        def out_stage(lp, wt, p_h):
            # out[lp] = W @ P_flat: paired-bank PSUM, 2 half stores.
            # Inputs were finished during the previous iteration, so these
            # matmuls/copies fill every engine's dependency stalls while
            # the CURRENT layer's selection chain is still in flight.
            for h in range(2):
                ob = obp.tile([BS, NF // 2], f32)
                for j in range(3):
                    po = ps_o.tile([BS, 1024], f32)
                    for g in range(2):
                        n = j * 2 + g
                        nc.tensor.matmul(
                            po[:, g * 512 : (g + 1) * 512],
                            wt[:],
                            p_h[h][:, n * 512 : (n + 1) * 512],
                            start=True,
                            stop=True,
                        )
                    if j == 2:
                        nc.scalar.copy(ob[:, j * 1024 : (j + 1) * 1024], po[:])
                    else:
                        nc.vector.tensor_copy(
                            ob[:, j * 1024 : (j + 1) * 1024], po[:]
                        )
                nc.scalar.dma_start(
                    o_d[lp][:, h * (NF // 2) : (h + 1) * (NF // 2)], ob[:]
                )

        prev = None
        for l in range(L):
            # ---- all loads issue from the SP ring (nc.sync) ----
            kat = katp.tile([128, 2 * NC6 * P_N], f32)
            nc.sync.dma_start(kat[:], kat_d[l])
            xt = xtp.tile([BS, D], f32)
            nc.sync.dma_start(xt[:], xt_d[l])
            p_h = []
            for h in range(2):
                ph = ppool.tile([P_N, NF // 2], f32r, tag="ph")
                nc.sync.dma_start(ph[:], p_d[l, h])
                p_h.append(ph)

            # ---- previous layer's output stage (software pipelining) ----
            if prev is not None:
                out_stage(l - 1, *prev)

            # ---- partial products: K^2 | A^2 | K*A  (d on partitions) ----
            sq = sqp.tile([128, 1800], f32)
            nc.scalar.activation(sq[:, 0:600], kat[:, 0:600], AF.Square)
            nc.scalar.activation(sq[:, 600:1200], kat[:, 600:1200], AF.Square)
            nc.vector.tensor_tensor(
                sq[:, 1200:1800], kat[:, 0:600], kat[:, 600:1200], op=OP.mult
            )

            # ---- fold the 6 d-chunks of each quantity: [128,1800]->[128,300]
            tq = foldp.tile([128, 900], f32, tag="tq")
            f300 = foldp.tile([128, 300], f32, tag="f300")
            sq_v = sq[:].rearrange("p (q c j) -> p q c j", q=3, c=6)
            tq_v = tq[:].rearrange("p (q c j) -> p q c j", q=3, c=3)
            f_v = f300[:].rearrange("p (q j) -> p q j", q=3)
            nc.vector.tensor_tensor(tq_v, sq_v[:, :, 0:3], sq_v[:, :, 3:6], op=OP.add)
            nc.vector.tensor_tensor(f_v, tq_v[:, :, 0], tq_v[:, :, 1], op=OP.add)
            nc.vector.tensor_tensor(f_v, f_v, tq_v[:, :, 2], op=OP.add)

            # ---- partition-dim reduce: 3 matmuls into [100,4] columns ----
            red = ps_sp.tile([P_N, 4], f32, tag="sp")
            for q in range(3):
                nc.tensor.matmul(
                    red[:, q : q + 1],
                    f300[:, q * 100 : (q + 1) * 100],
                    ones_col[:],
                    start=True,
                    stop=True,
                )
            sqs = smp.tile([P_N, 3], f32, tag="sqs")
            nc.scalar.copy(sqs[:], red[:, 0:3])

            # ---- rsqrt of ||K||^2, ||A||^2 with one Newton step ----
            srt = smp.tile([P_N, 2], f32, tag="srt")
            nc.scalar.activation(srt[:], sqs[:, 0:2], AF.Sqrt)
            y0 = smp.tile([P_N, 2], f32, tag="y0")
            nc.vector.reciprocal(y0[:], srt[:])
            t1 = smp.tile([P_N, 2], f32, tag="t1")
            nc.vector.tensor_tensor(t1[:], y0[:], y0[:], op=OP.mult)
            nc.vector.tensor_tensor(t1[:], t1[:], sqs[:, 0:2], op=OP.mult)
            nc.vector.tensor_scalar(t1[:], t1[:], -0.5, 1.5, OP.mult, OP.add)
            rs2 = smp.tile([P_N, 2], f32, tag="rs2")
            nc.vector.tensor_tensor(rs2[:], t1[:], y0[:], op=OP.mult)

            # s[p] = <K,A> * rsK * rsA   (column form, for the wt scale)
            s_col = smp.tile([P_N, 1], f32, tag="scol")
            nc.vector.tensor_tensor(s_col[:], rs2[:, 0:1], rs2[:, 1:2], op=OP.mult)
            nc.vector.tensor_tensor(s_col[:], s_col[:], sqs[:, 2:3], op=OP.mult)

            # ---- scores transposed: [100p, 128b] = sum_c K_c.T @ x_c ----
            scT = ps_sp.tile([P_N, BS], f32, tag="sp")
            for c in range(NC6):
                nc.tensor.matmul(
                    scT[:],
                    kat[:, c * P_N : (c + 1) * P_N],
                    xt[:, c * 128 : (c + 1) * 128],
                    start=(c == 0),
                    stop=(c == NC6 - 1),
                )
            su = smp.tile([P_N, BS], f32, tag="su")
            nc.vector.tensor_scalar_mul(su[:], scT[:], rs2[:, 0:1])

            # back to [128b, 100p] for the row top-k
            St = ps_sp.tile([BS, P_N], f32, tag="sp")
            nc.tensor.transpose(St[:], su[:], ident[:P_N, :P_N])
            Ssb = smp.tile([BS, P_N], f32, tag="Ssb")
            nc.scalar.copy(Ssb[:], St[:])

            # ---- top-8 per row in one DVE op; threshold = 5th largest ----
            m8 = smp.tile([BS, 8], f32, tag="m8")
            nc.vector.max(m8[:], Ssb[:])
            mask = smp.tile([BS, P_N], f32, tag="mask")
            nc.vector.tensor_scalar(
                mask[:], Ssb[:], m8[:, TOP_K - 1 : TOP_K], None, OP.is_ge
            )

            # W^T = mask^T * s -> [100, 128]
            mt = ps_sp.tile([P_N, BS], f32, tag="sp")
            nc.tensor.transpose(mt[:], mask[:], ident[:])
            wt = smp.tile([P_N, BS], f32r, tag="wt")
            nc.vector.tensor_scalar_mul(wt[:], mt[:], s_col[:])

            prev = (wt, p_h)

        out_stage(L - 1, *prev)

    nc.compile()
    _CACHE["nc"] = nc
    return nc


def _pack_inputs(x_query, K_all, A_all, P_all):
    x = np.ascontiguousarray(np.asarray(x_query, dtype=np.float32))
    k = np.asarray(K_all, dtype=np.float32)
    a = np.asarray(A_all, dtype=np.float32)
    p = np.ascontiguousarray(
        np.asarray(P_all, dtype=np.float32)
        .reshape(L, P_N, 2, NF // 2)
        .transpose(0, 2, 1, 3)
    )

    def t_pool(m):  # [L,P,D] -> [L,128,6*P]: out[l,p,c*P+j] = m[l,j,c*128+p]
        r = m.transpose(0, 2, 1).reshape(L, NC6, 128, P_N)
        return r.transpose(0, 2, 1, 3).reshape(L, 128, NC6 * P_N)

    kat = np.ascontiguousarray(np.concatenate([t_pool(k), t_pool(a)], axis=2))

    xts = []
    for c in range(N_CORES):
        xc = x[c * BS : (c + 1) * BS]  # [128, L, D]
        # xt[l,p,c6*128+b] = xc[b,l,c6*128+p]
        r = xc.transpose(1, 2, 0).reshape(L, NC6, 128, BS)
        xts.append(
            np.ascontiguousarray(r.transpose(0, 2, 1, 3).reshape(L, BS, D))
        )
    return xts, kat, p


def _run(x_query, K_all, A_all, P_all, trace=False, tmpdir=None):
    from concourse.bass_utils import run_bass_kernel_spmd

    xts, kat, p = _pack_inputs(x_query, K_all, A_all, P_all)

    nc = _build_nc()
    in_maps = [{"xt": xts[c], "kat": kat, "p": p} for c in range(N_CORES)]
    br = run_bass_kernel_spmd(
        nc, in_maps, list(range(N_CORES)), trace=trace, tmpdir=tmpdir
    )
    out = np.stack([r["o"] for r in br.results], axis=0)  # [8, L, BS, NF]
    out = out.transpose(1, 0, 2, 3).reshape(L, B, LP, D)
    return out, br


def kernel(x_query, K_all, A_all, P_all):
    out, _ = _run(x_query, K_all, A_all, P_all)
    return out


# revision 9
# speedup vs baseline: 1.1482x; 1.0390x over previous
"""Trainium2 Bass kernel for nn_L2PppMaskAttn (topk_masking).

Math reformulation of the reference:
  - a_k = sum(l2norm(K[idx]) * l2norm(A[idx])) depends only on (layer, prompt):
    precompute s[l,p] = <K,A> / (||K|| ||A||) once per layer on-device.
  - top-5 ranking over prompts is invariant to q normalization (positive
    per-row scale), so scores u[b,p] = <x[b,l], K[l,p]> / ||K[l,p]|| suffice.
  - out[l,b] = sum_{p in top5} s[l,p] * P[l,p] = (mask_row .* s) @ P_flat[l],
    a dense [B,100] @ [100, 6144] matmul per layer (topk -> masking).

v5 notes (356us baseline -> targeting the ~222us memory roofline):
  - Host passes x, K, A pre-transposed (d on partitions).
  - ||K||^2/||A||^2/<K,A>: square/mult partials, strided 3-op DVE chunk
    fold, then only 3 ones-moving matmuls into a [100,4] column block
    (fp32-stationary matmuls double-emit HI/LO, so keep PE count low).
  - Norm chain entirely in [100,k] column form (single-partition DVE ops
    are ~7x slower; 1-lane only).
  - Scores computed TRANSPOSED so the 1/||K|| scale is a native
    per-partition tensor_scalar; one PE transpose back for the top-k.
  - top-5 threshold via the DVE InstMax top-8 instruction (1 op).
  - DMA triggers must not block compute engines: ALL loads issue from
    nc.sync (pure DMA engine; slot waits harmless), stores from
    nc.scalar (ACT reaches the trigger right after producing ob, so the
    wait is ~0). GpSimd/SWDGE unused.
  - Output matmul pairs share a [128,1024] 2-bank PSUM tile -> 6 wider
    PSUM->SBUF copies per layer instead of 12.

Sharding: data-parallel over batch, 8 cores x 128 rows; K/A/P replicated.
"""

import sys

sys.path.insert(0, "/opt/trn_rl_repo")

import numpy as np

B, L, P_N, LP, D = 1024, 12, 100, 8, 768
N_CORES = 8
BS = B // N_CORES  # 128 batch rows per core
NF = LP * D  # 6144 flattened output features per layer
NC6 = D // 128  # 6 contraction chunks of 128
TOP_K = 5

_CACHE = {}


def _build_nc():
    if "nc" in _CACHE:
        return _CACHE["nc"]

    from contextlib import ExitStack

    import concourse.bass as bass
    import concourse.bacc as bacc
    import concourse.mybir as mybir
    from concourse import masks
    from concourse.tile import TileContext

    f32 = mybir.dt.float32
    f32r = mybir.dt.float32r
    AX = mybir.AxisListType
    OP = mybir.AluOpType
    AF = mybir.ActivationFunctionType

    nc = bacc.Bacc(
        "TRN2",
        target_bir_lowering=False,
        debug=False,
        num_devices=N_CORES,
    )

    # Host-side layouts (see _pack_inputs):
    #  xt[l, p, c*128 + b]        = x_core[b, l, c*128 + p]
    #  kat[l, p, c*100 + j]       = K[l, j, c*128 + p]      (cols 0..599)
    #  kat[l, p, 600 + c*100 + j] = A[l, j, c*128 + p]      (cols 600..1199)
    #  p[l, j, :]                 = P[l, j].reshape(NF)
    xt_d = nc.declare_dram_parameter("xt", [L, BS, D], f32, isOutput=False)
    kat_d = nc.declare_dram_parameter(
        "kat", [L, 128, 2 * NC6 * P_N], f32, isOutput=False
    )
    p_d = nc.declare_dram_parameter("p", [L, 2, P_N, NF // 2], f32r, isOutput=False)
    o_d = nc.declare_dram_parameter("o", [L, BS, NF], f32, isOutput=True)

    with TileContext(nc) as tc, ExitStack() as ctx:
        pool = lambda name, bufs, **kw: ctx.enter_context(
            tc.tile_pool(name=name, bufs=bufs, **kw)
        )
        const = pool("const", 1)
        katp = pool("katp", 6)
        xtp = pool("xtp", 6)
        ppool = pool("pp", 6)
        sqp = pool("sqp", 2)
        foldp = pool("foldp", 2)
        smp = pool("smp", 3)
        obp = pool("obp", 4)
        ps_sp = pool("ps_sp", 4, space="PSUM")  # red/scT/St/mt share one tag
        ps_o = pool("ps_o", 2, space="PSUM")    # [128,1024] = 2 banks each

        ident = const.tile([128, 128], f32)
        masks.make_identity(nc, ident[:])
        ones_col = const.tile([128, 1], f32)
        nc.vector.memset(ones_col[:], 1.0)

        def out_stage(lp, wt, p_h):
            # out[lp] = W @ P_flat: paired-bank PSUM, 2 half stores.
            # Inputs were finished during the previous iteration, so these
            # matmuls/copies fill every engine's dependency stalls while
            # the CURRENT layer's selection chain is still in flight.
            for h in range(2):
                ob = obp.tile([BS, NF // 2], f32)
                for j in range(3):
                    po = ps_o.tile([BS, 1024], f32)
                    for g in range(2):
                        n = j * 2 + g
                        nc.tensor.matmul(
                            po[:, g * 512 : (g + 1) * 512],
                            wt[:],
                            p_h[h][:, n * 512 : (n + 1) * 512],
                            start=True,
                            stop=True,
                        )
                    if j == 0:
                        nc.vector.tensor_copy(
                            ob[:, j * 1024 : (j + 1) * 1024], po[:]
                        )
                    else:
                        nc.scalar.copy(ob[:, j * 1024 : (j + 1) * 1024], po[:])
                nc.gpsimd.dma_start(
                    o_d[lp][:, h * (NF // 2) : (h + 1) * (NF // 2)], ob[:]
                )

        prev = None
        for l in range(L):
            # ---- all loads issue from the SP ring (nc.sync) ----
            kat = katp.tile([128, 2 * NC6 * P_N], f32)
            nc.sync.dma_start(kat[:], kat_d[l])
            xt = xtp.tile([BS, D], f32)
            nc.sync.dma_start(xt[:], xt_d[l])
            p_h = []
            for h in range(2):
                ph = ppool.tile([P_N, NF // 2], f32r, tag="ph")
                nc.sync.dma_start(ph[:], p_d[l, h])
                p_h.append(ph)

            # ---- previous layer's output stage (software pipelining) ----
            if prev is not None:
                out_stage(l - 1, *prev)

            # ---- partial products: K^2 | A^2 | K*A  (d on partitions) ----
            sq = sqp.tile([128, 1800], f32)
            nc.scalar.activation(sq[:, 0:600], kat[:, 0:600], AF.Square)
            nc.scalar.activation(sq[:, 600:1200], kat[:, 600:1200], AF.Square)
            nc.vector.tensor_tensor(
                sq[:, 1200:1800], kat[:, 0:600], kat[:, 600:1200], op=OP.mult
            )

            # ---- fold the 6 d-chunks of each quantity: [128,1800]->[128,300]
            tq = foldp.tile([128, 900], f32, tag="tq")
            f300 = foldp.tile([128, 300], f32, tag="f300")
            sq_v = sq[:].rearrange("p (q c j) -> p q c j", q=3, c=6)
            tq_v = tq[:].rearrange("p (q c j) -> p q c j", q=3, c=3)
            f_v = f300[:].rearrange("p (q j) -> p q j", q=3)
            nc.vector.tensor_tensor(tq_v, sq_v[:, :, 0:3], sq_v[:, :, 3:6], op=OP.add)
            nc.vector.tensor_tensor(f_v, tq_v[:, :, 0], tq_v[:, :, 1], op=OP.add)
            nc.vector.tensor_tensor(f_v, f_v, tq_v[:, :, 2], op=OP.add)

            # ---- partition-dim reduce: 3 matmuls into [100,4] columns ----
            red = ps_sp.tile([P_N, 4], f32, tag="sp")
            for q in range(3):
                nc.tensor.matmul(
                    red[:, q : q + 1],
                    f300[:, q * 100 : (q + 1) * 100],
                    ones_col[:],
                    start=True,
                    stop=True,
                )
            sqs = smp.tile([P_N, 3], f32, tag="sqs")
            nc.scalar.copy(sqs[:], red[:, 0:3])

            # ---- rsqrt of ||K||^2, ||A||^2 with one Newton step ----
            srt = smp.tile([P_N, 2], f32, tag="srt")
            nc.scalar.activation(srt[:], sqs[:, 0:2], AF.Sqrt)
            y0 = smp.tile([P_N, 2], f32, tag="y0")
            nc.vector.reciprocal(y0[:], srt[:])
            t1 = smp.tile([P_N, 2], f32, tag="t1")
            nc.vector.tensor_tensor(t1[:], y0[:], y0[:], op=OP.mult)
            nc.vector.tensor_tensor(t1[:], t1[:], sqs[:, 0:2], op=OP.mult)
            nc.vector.tensor_scalar(t1[:], t1[:], -0.5, 1.5, OP.mult, OP.add)
            rs2 = smp.tile([P_N, 2], f32, tag="rs2")
            nc.vector.tensor_tensor(rs2[:], t1[:], y0[:], op=OP.mult)

            # s[p] = <K,A> * rsK * rsA   (column form, for the wt scale)
            s_col = smp.tile([P_N, 1], f32, tag="scol")
            nc.vector.tensor_tensor(s_col[:], rs2[:, 0:1], rs2[:, 1:2], op=OP.mult)
            nc.vector.tensor_tensor(s_col[:], s_col[:], sqs[:, 2:3], op=OP.mult)

            # ---- scores transposed: [100p, 128b] = sum_c K_c.T @ x_c ----
            scT = ps_sp.tile([P_N, BS], f32, tag="sp")
            for c in range(NC6):
                nc.tensor.matmul(
                    scT[:],
                    kat[:, c * P_N : (c + 1) * P_N],
                    xt[:, c * 128 : (c + 1) * 128],
                    start=(c == 0),
                    stop=(c == NC6 - 1),
                )
            su = smp.tile([P_N, BS], f32, tag="su")
            nc.vector.tensor_scalar_mul(su[:], scT[:], rs2[:, 0:1])

            # back to [128b, 100p] for the row top-k
            St = ps_sp.tile([BS, P_N], f32, tag="sp")
            nc.tensor.transpose(St[:], su[:], ident[:P_N, :P_N])
            Ssb = smp.tile([BS, P_N], f32, tag="Ssb")
            nc.vector.tensor_copy(Ssb[:], St[:])

            # ---- top-8 per row in one DVE op; threshold = 5th largest ----
            m8 = smp.tile([BS, 8], f32, tag="m8")
            nc.vector.max(m8[:], Ssb[:])
            mask = smp.tile([BS, P_N], f32, tag="mask")
            nc.vector.tensor_scalar(
                mask[:], Ssb[:], m8[:, TOP_K - 1 : TOP_K], None, OP.is_ge
            )

            # W^T = mask^T * s -> [100, 128]
            mt = ps_sp.tile([P_N, BS], f32, tag="sp")
            nc.tensor.transpose(mt[:], mask[:], ident[:])
            wt = smp.tile([P_N, BS], f32r, tag="wt")
            nc.vector.tensor_scalar_mul(wt[:], mt[:], s_col[:])

            prev = (wt, p_h)

        out_stage(L - 1, *prev)

    nc.compile()
    _CACHE["nc"] = nc
    return nc


def _pack_inputs(x_query, K_all, A_all, P_all):
    x = np.ascontiguousarray(np.asarray(x_query, dtype=np.float32))
    k = np.asarray(K_all, dtype=np.float32)
    a = np.asarray(A_all, dtype=np.float32)
    p = np.ascontiguousarray(
        np.asarray(P_all, dtype=np.float32)
        .reshape(L, P_N, 2, NF // 2)
        .transpose(0, 2, 1, 3)
    )

    def t_pool(m):  # [L,P,D] -> [L,128,6*P]: out[l,p,c*P+j] = m[l,j,c*128+p]
        r = m.transpose(0, 2, 1).reshape(L, NC6, 128, P_N)
        return r.transpose(0, 2, 1, 3).reshape(L, 128, NC6 * P_N)

    kat = np.ascontiguousarray(np.concatenate([t_pool(k), t_pool(a)], axis=2))

    xts = []
    for c in range(N_CORES):
        xc = x[c * BS : (c + 1) * BS]  # [128, L, D]
        # xt[l,p,c6*128+b] = xc[b,l,c6*128+p]
        r = xc.transpose(1, 2, 0).reshape(L, NC6, 128, BS)
        xts.append(
            np.ascontiguousarray(r.transpose(0, 2, 1, 3).reshape(L, BS, D))
        )
    return xts, kat, p


def _run(x_query, K_all, A_all, P_all, trace=False, tmpdir=None):
    from concourse.bass_utils import run_bass_kernel_spmd

    xts, kat, p = _pack_inputs(x_query, K_all, A_all, P_all)

    nc = _build_nc()
    in_maps = [{"xt": xts[c], "kat": kat, "p": p} for c in range(N_CORES)]
    br = run_bass_kernel_spmd(
        nc, in_maps, list(range(N_CORES)), trace=trace, tmpdir=tmpdir
    )
    out = np.stack([r["o"] for r in br.results], axis=0)  # [8, L, BS, NF]
    out = out.transpose(1, 0, 2, 3).reshape(L, B, LP, D)
    return out, br


def kernel(x_query, K_all, A_all, P_all):
    out, _ = _run(x_query, K_all, A_all, P_all)
    return out


# revision 10
# speedup vs baseline: 1.1986x; 1.0439x over previous
"""Trainium2 Bass kernel for nn_L2PppMaskAttn (topk_masking).

Math reformulation of the reference:
  - a_k = sum(l2norm(K[idx]) * l2norm(A[idx])) depends only on (layer, prompt):
    precompute s[l,p] = <K,A> / (||K|| ||A||) once per layer on-device.
  - top-5 ranking over prompts is invariant to q normalization (positive
    per-row scale), so scores u[b,p] = <x[b,l], K[l,p]> / ||K[l,p]|| suffice.
  - out[l,b] = sum_{p in top5} s[l,p] * P[l,p] = (mask_row .* s) @ P_flat[l],
    a dense [B,100] @ [100, 6144] matmul per layer (topk -> masking).

v5 notes (356us baseline -> targeting the ~222us memory roofline):
  - Host passes x, K, A pre-transposed (d on partitions).
  - ||K||^2/||A||^2/<K,A>: square/mult partials, strided 3-op DVE chunk
    fold, then only 3 ones-moving matmuls into a [100,4] column block
    (fp32-stationary matmuls double-emit HI/LO, so keep PE count low).
  - Norm chain entirely in [100,k] column form (single-partition DVE ops
    are ~7x slower; 1-lane only).
  - Scores computed TRANSPOSED so the 1/||K|| scale is a native
    per-partition tensor_scalar; one PE transpose back for the top-k.
  - top-5 threshold via the DVE InstMax top-8 instruction (1 op).
  - DMA triggers must not block compute engines: ALL loads issue from
    nc.sync (pure DMA engine; slot waits harmless), stores from
    nc.scalar (ACT reaches the trigger right after producing ob, so the
    wait is ~0). GpSimd/SWDGE unused.
  - Output matmul pairs share a [128,1024] 2-bank PSUM tile -> 6 wider
    PSUM->SBUF copies per layer instead of 12.

Sharding: data-parallel over batch, 8 cores x 128 rows; K/A/P replicated.
"""

import sys

sys.path.insert(0, "/opt/trn_rl_repo")

import numpy as np

B, L, P_N, LP, D = 1024, 12, 100, 8, 768
N_CORES = 8
BS = B // N_CORES  # 128 batch rows per core
NF = LP * D  # 6144 flattened output features per layer
NC6 = D // 128  # 6 contraction chunks of 128
TOP_K = 5

_CACHE = {}


def _build_nc():
    if "nc" in _CACHE:
        return _CACHE["nc"]

    from contextlib import ExitStack

    import concourse.bass as bass
    import concourse.bacc as bacc
    import concourse.mybir as mybir
    from concourse import masks
    from concourse.tile import TileContext

    f32 = mybir.dt.float32
    f32r = mybir.dt.float32r
    AX = mybir.AxisListType
    OP = mybir.AluOpType
    AF = mybir.ActivationFunctionType

    nc = bacc.Bacc(
        "TRN2",
        target_bir_lowering=False,
        debug=False,
        num_devices=N_CORES,
    )

    # Host-side layouts (see _pack_inputs):
    #  xt[l, p, c*128 + b]        = x_core[b, l, c*128 + p]
    #  kat[l, p, c*100 + j]       = K[l, j, c*128 + p]      (cols 0..599)
    #  kat[l, p, 600 + c*100 + j] = A[l, j, c*128 + p]      (cols 600..1199)
    #  p[l, j, :]                 = P[l, j].reshape(NF)
    xt_d = nc.declare_dram_parameter("xt", [L, BS, D], f32, isOutput=False)
    kat_d = nc.declare_dram_parameter(
        "kat", [L, 128, 2 * NC6 * P_N], f32, isOutput=False
    )
    p_d = nc.declare_dram_parameter("p", [L, 2, P_N, NF // 2], f32r, isOutput=False)
    o_d = nc.declare_dram_parameter("o", [L, BS, NF], f32, isOutput=True)

    with TileContext(nc) as tc, ExitStack() as ctx:
        pool = lambda name, bufs, **kw: ctx.enter_context(
            tc.tile_pool(name=name, bufs=bufs, **kw)
        )
        const = pool("const", 1)
        katp = pool("katp", 4)
        xtp = pool("xtp", 4)
        ppool = pool("pp", 8)
        sqp = pool("sqp", 2)
        foldp = pool("foldp", 2)
        smp = pool("smp", 3)
        obp = pool("obp", 3)
        ps_sp = pool("ps_sp", 4, space="PSUM")  # red/scT/St/mt share one tag
        ps_o = pool("ps_o", 2, space="PSUM")    # [128,1024] = 2 banks each

        ident = const.tile([128, 128], f32)
        masks.make_identity(nc, ident[:])
        ones_col = const.tile([128, 1], f32)
        nc.vector.memset(ones_col[:], 1.0)

        def out_stage(lp, wt, p_h):
            # out[lp] = W @ P_flat: paired-bank PSUM, 2 half stores.
            # Inputs were finished during the previous iteration, so these
            # matmuls/copies fill every engine's dependency stalls while
            # the CURRENT layer's selection chain is still in flight.
            for h in range(2):
                ob = obp.tile([BS, NF // 2], f32)
                for j in range(3):
                    po = ps_o.tile([BS, 1024], f32)
                    for g in range(2):
                        n = j * 2 + g
                        nc.tensor.matmul(
                            po[:, g * 512 : (g + 1) * 512],
                            wt[:],
                            p_h[h][:, n * 512 : (n + 1) * 512],
                            start=True,
                            stop=True,
                        )
                    if j == 0:
                        nc.vector.tensor_copy(
                            ob[:, j * 1024 : (j + 1) * 1024], po[:]
                        )
                    else:
                        nc.scalar.copy(ob[:, j * 1024 : (j + 1) * 1024], po[:])
                nc.gpsimd.dma_start(
                    o_d[lp][:, h * (NF // 2) : (h + 1) * (NF // 2)], ob[:]
                )

        prev = None
        for l in range(L):
            # ---- all loads issue from the SP ring (nc.sync) ----
            kat = katp.tile([128, 2 * NC6 * P_N], f32)
            nc.sync.dma_start(kat[:], kat_d[l])
            xt = xtp.tile([BS, D], f32)
            nc.sync.dma_start(xt[:], xt_d[l])
            p_h = []
            for h in range(2):
                ph = ppool.tile([P_N, NF // 2], f32r, tag="ph")
                nc.sync.dma_start(ph[:], p_d[l, h])
                p_h.append(ph)

            # ---- previous layer's output stage (software pipelining) ----
            if prev is not None:
                out_stage(l - 1, *prev)

            # ---- partial products: K^2 | A^2 | K*A  (d on partitions) ----
            sq = sqp.tile([128, 1800], f32)
            nc.scalar.activation(sq[:, 0:600], kat[:, 0:600], AF.Square)
            nc.scalar.activation(sq[:, 600:1200], kat[:, 600:1200], AF.Square)
            nc.vector.tensor_tensor(
                sq[:, 1200:1800], kat[:, 0:600], kat[:, 600:1200], op=OP.mult
            )

            # ---- fold the 6 d-chunks of each quantity: [128,1800]->[128,300]
            tq = foldp.tile([128, 900], f32, tag="tq")
            f300 = foldp.tile([128, 300], f32, tag="f300")
            sq_v = sq[:].rearrange("p (q c j) -> p q c j", q=3, c=6)
            tq_v = tq[:].rearrange("p (q c j) -> p q c j", q=3, c=3)
            f_v = f300[:].rearrange("p (q j) -> p q j", q=3)
            nc.vector.tensor_tensor(tq_v, sq_v[:, :, 0:3], sq_v[:, :, 3:6], op=OP.add)
            nc.vector.tensor_tensor(f_v, tq_v[:, :, 0], tq_v[:, :, 1], op=OP.add)
            nc.vector.tensor_tensor(f_v, f_v, tq_v[:, :, 2], op=OP.add)

            # ---- partition-dim reduce: 3 matmuls into [100,4] columns ----
            red = ps_sp.tile([P_N, 4], f32, tag="sp")
            for q in range(3):
                nc.tensor.matmul(
                    red[:, q : q + 1],
                    f300[:, q * 100 : (q + 1) * 100],
                    ones_col[:],
                    start=True,
                    stop=True,
                )
            sqs = smp.tile([P_N, 3], f32, tag="sqs")
            nc.scalar.copy(sqs[:], red[:, 0:3])

            # ---- rsqrt of ||K||^2, ||A||^2 with one Newton step ----
            srt = smp.tile([P_N, 2], f32, tag="srt")
            nc.scalar.activation(srt[:], sqs[:, 0:2], AF.Sqrt)
            y0 = smp.tile([P_N, 2], f32, tag="y0")
            nc.vector.reciprocal(y0[:], srt[:])
            t1 = smp.tile([P_N, 2], f32, tag="t1")
            nc.vector.tensor_tensor(t1[:], y0[:], y0[:], op=OP.mult)
            nc.vector.tensor_tensor(t1[:], t1[:], sqs[:, 0:2], op=OP.mult)
            nc.vector.tensor_scalar(t1[:], t1[:], -0.5, 1.5, OP.mult, OP.add)
            rs2 = smp.tile([P_N, 2], f32, tag="rs2")
            nc.vector.tensor_tensor(rs2[:], t1[:], y0[:], op=OP.mult)

            # s[p] = <K,A> * rsK * rsA   (column form, for the wt scale)
            s_col = smp.tile([P_N, 1], f32, tag="scol")
            nc.vector.tensor_tensor(s_col[:], rs2[:, 0:1], rs2[:, 1:2], op=OP.mult)
            nc.vector.tensor_tensor(s_col[:], s_col[:], sqs[:, 2:3], op=OP.mult)

            # ---- scores transposed: [100p, 128b] = sum_c K_c.T @ x_c ----
            scT = ps_sp.tile([P_N, BS], f32, tag="sp")
            for c in range(NC6):
                nc.tensor.matmul(
                    scT[:],
                    kat[:, c * P_N : (c + 1) * P_N],
                    xt[:, c * 128 : (c + 1) * 128],
                    start=(c == 0),
                    stop=(c == NC6 - 1),
                )
            su = smp.tile([P_N, BS], f32, tag="su")
            nc.vector.tensor_scalar_mul(su[:], scT[:], rs2[:, 0:1])

            # back to [128b, 100p] for the row top-k
            St = ps_sp.tile([BS, P_N], f32, tag="sp")
            nc.tensor.transpose(St[:], su[:], ident[:P_N, :P_N])
            Ssb = smp.tile([BS, P_N], f32, tag="Ssb")
            nc.vector.tensor_copy(Ssb[:], St[:])

            # ---- top-8 per row in one DVE op; threshold = 5th largest ----
            m8 = smp.tile([BS, 8], f32, tag="m8")
            nc.vector.max(m8[:], Ssb[:])
            mask = smp.tile([BS, P_N], f32, tag="mask")
            nc.vector.tensor_scalar(
                mask[:], Ssb[:], m8[:, TOP_K - 1 : TOP_K], None, OP.is_ge
            )

            # W^T = mask^T * s -> [100, 128]
            mt = ps_sp.tile([P_N, BS], f32, tag="sp")
            nc.tensor.transpose(mt[:], mask[:], ident[:])
            wt = smp.tile([P_N, BS], f32r, tag="wt")
            nc.vector.tensor_scalar_mul(wt[:], mt[:], s_col[:])

            prev = (wt, p_h)

        out_stage(L - 1, *prev)

    nc.compile()
    _CACHE["nc"] = nc
    return nc


def _pack_inputs(x_query, K_all, A_all, P_all):
    x = np.ascontiguousarray(np.asarray(x_query, dtype=np.float32))
    k = np.asarray(K_all, dtype=np.float32)
    a = np.asarray(A_all, dtype=np.float32)
    p = np.ascontiguousarray(
        np.asarray(P_all, dtype=np.float32)
        .reshape(L, P_N, 2, NF // 2)
        .transpose(0, 2, 1, 3)
    )

    def t_pool(m):  # [L,P,D] -> [L,128,6*P]: out[l,p,c*P+j] = m[l,j,c*128+p]
        r = m.transpose(0, 2, 1).reshape(L, NC6, 128, P_N)
        return r.transpose(0, 2, 1, 3).reshape(L, 128, NC6 * P_N)

    kat = np.ascontiguousarray(np.concatenate([t_pool(k), t_pool(a)], axis=2))

    xts = []
    for c in range(N_CORES):
        xc = x[c * BS : (c + 1) * BS]  # [128, L, D]
        # xt[l,p,c6*128+b] = xc[b,l,c6*128+p]
        r = xc.transpose(1, 2, 0).reshape(L, NC6, 128, BS)
        xts.append(
            np.ascontiguousarray(r.transpose(0, 2, 1, 3).reshape(L, BS, D))
        )
    return xts, kat, p


def _run(x_query, K_all, A_all, P_all, trace=False, tmpdir=None):
    from concourse.bass_utils import run_bass_kernel_spmd

    xts, kat, p = _pack_inputs(x_query, K_all, A_all, P_all)

    nc = _build_nc()
    in_maps = [{"xt": xts[c], "kat": kat, "p": p} for c in range(N_CORES)]
    br = run_bass_kernel_spmd(
        nc, in_maps, list(range(N_CORES)), trace=trace, tmpdir=tmpdir
    )
    out = np.stack([r["o"] for r in br.results], axis=0)  # [8, L, BS, NF]
    out = out.transpose(1, 0, 2, 3).reshape(L, B, LP, D)
    return out, br


def kernel(x_query, K_all, A_all, P_all):
    out, _ = _run(x_query, K_all, A_all, P_all)
    return out


# revision 11
# speedup vs baseline: 1.2201x; 1.0180x over previous
"""Trainium2 Bass kernel for nn_L2PppMaskAttn (topk_masking).

Math reformulation of the reference:
  - a_k = sum(l2norm(K[idx]) * l2norm(A[idx])) depends only on (layer, prompt):
    precompute s[l,p] = <K,A> / (||K|| ||A||) once per layer on-device.
  - top-5 ranking over prompts is invariant to q normalization (positive
    per-row scale), so scores u[b,p] = <x[b,l], K[l,p]> / ||K[l,p]|| suffice.
  - out[l,b] = sum_{p in top5} s[l,p] * P[l,p] = (mask_row .* s) @ P_flat[l],
    a dense [B,100] @ [100, 6144] matmul per layer (topk -> masking).

v5 notes (356us baseline -> targeting the ~222us memory roofline):
  - Host passes x, K, A pre-transposed (d on partitions).
  - ||K||^2/||A||^2/<K,A>: square/mult partials, strided 3-op DVE chunk
    fold, then only 3 ones-moving matmuls into a [100,4] column block
    (fp32-stationary matmuls double-emit HI/LO, so keep PE count low).
  - Norm chain entirely in [100,k] column form (single-partition DVE ops
    are ~7x slower; 1-lane only).
  - Scores computed TRANSPOSED so the 1/||K|| scale is a native
    per-partition tensor_scalar; one PE transpose back for the top-k.
  - top-5 threshold via the DVE InstMax top-8 instruction (1 op).
  - DMA triggers must not block compute engines: ALL loads issue from
    nc.sync (pure DMA engine; slot waits harmless), stores from
    nc.scalar (ACT reaches the trigger right after producing ob, so the
    wait is ~0). GpSimd/SWDGE unused.
  - Output matmul pairs share a [128,1024] 2-bank PSUM tile -> 6 wider
    PSUM->SBUF copies per layer instead of 12.

Sharding: data-parallel over batch, 8 cores x 128 rows; K/A/P replicated.
"""

import sys

sys.path.insert(0, "/opt/trn_rl_repo")

import numpy as np

B, L, P_N, LP, D = 1024, 12, 100, 8, 768
N_CORES = 8
BS = B // N_CORES  # 128 batch rows per core
NF = LP * D  # 6144 flattened output features per layer
NC6 = D // 128  # 6 contraction chunks of 128
TOP_K = 5

_CACHE = {}


def _build_nc():
    if "nc" in _CACHE:
        return _CACHE["nc"]

    from contextlib import ExitStack

    import concourse.bass as bass
    import concourse.bacc as bacc
    import concourse.mybir as mybir
    from concourse import masks
    from concourse.tile import TileContext

    f32 = mybir.dt.float32
    f32r = mybir.dt.float32r
    AX = mybir.AxisListType
    OP = mybir.AluOpType
    AF = mybir.ActivationFunctionType

    nc = bacc.Bacc(
        "TRN2",
        target_bir_lowering=False,
        debug=False,
        num_devices=N_CORES,
    )

    # Host-side layouts (see _pack_inputs):
    #  xt[l, p, c*128 + b]        = x_core[b, l, c*128 + p]
    #  kat[l, p, c*100 + j]       = K[l, j, c*128 + p]      (cols 0..599)
    #  kat[l, p, 600 + c*100 + j] = A[l, j, c*128 + p]      (cols 600..1199)
    #  p[l, j, :]                 = P[l, j].reshape(NF)
    xt_d = nc.declare_dram_parameter("xt", [L, BS, D], f32, isOutput=False)
    kat_d = nc.declare_dram_parameter(
        "kat", [L, 128, 2 * NC6 * P_N], f32, isOutput=False
    )
    p_d = nc.declare_dram_parameter("p", [L, 2, P_N, NF // 2], f32r, isOutput=False)
    o_d = nc.declare_dram_parameter("o", [L, BS, NF], f32, isOutput=True)

    with TileContext(nc) as tc, ExitStack() as ctx:
        pool = lambda name, bufs, **kw: ctx.enter_context(
            tc.tile_pool(name=name, bufs=bufs, **kw)
        )
        const = pool("const", 1)
        katp = pool("katp", 4)
        xtp = pool("xtp", 4)
        ppool = pool("pp", 8)
        sqp = pool("sqp", 2)
        foldp = pool("foldp", 2)
        smp = pool("smp", 3)
        obp = pool("obp", 3)
        ps_sp = pool("ps_sp", 4, space="PSUM")  # red/scT/St/mt share one tag
        ps_o = pool("ps_o", 2, space="PSUM")    # [128,1024] = 2 banks each

        ident = const.tile([128, 128], f32)
        masks.make_identity(nc, ident[:])
        ones_col = const.tile([128, 1], f32)
        nc.vector.memset(ones_col[:], 1.0)

        def out_stage(lp, wt, p_h):
            # out[lp] = W @ P_flat: paired-bank PSUM, 2 half stores.
            # Inputs were finished during the previous iteration, so these
            # matmuls/copies fill every engine's dependency stalls while
            # the CURRENT layer's selection chain is still in flight.
            for h in range(2):
                ob = obp.tile([BS, NF // 2], f32)
                for j in range(3):
                    po = ps_o.tile([BS, 1024], f32)
                    for g in range(2):
                        n = j * 2 + g
                        nc.tensor.matmul(
                            po[:, g * 512 : (g + 1) * 512],
                            wt[:],
                            p_h[h][:, n * 512 : (n + 1) * 512],
                            start=True,
                            stop=True,
                        )
                    if j == 0:
                        nc.vector.tensor_copy(
                            ob[:, j * 1024 : (j + 1) * 1024], po[:]
                        )
                    else:
                        nc.scalar.copy(ob[:, j * 1024 : (j + 1) * 1024], po[:])
                nc.gpsimd.dma_start(
                    o_d[lp][:, h * (NF // 2) : (h + 1) * (NF // 2)], ob[:]
                )

        prev = None
        for l in range(L):
            # ---- all loads issue from the SP ring (nc.sync) ----
            kat = katp.tile([128, 2 * NC6 * P_N], f32)
            nc.scalar.dma_start(kat[:], kat_d[l])
            xt = xtp.tile([BS, D], f32)
            nc.scalar.dma_start(xt[:], xt_d[l])
            p_h = []
            for h in range(2):
                ph = ppool.tile([P_N, NF // 2], f32r, tag="ph")
                nc.sync.dma_start(ph[:], p_d[l, h])
                p_h.append(ph)

            # ---- previous layer's output stage (software pipelining) ----
            if prev is not None:
                out_stage(l - 1, *prev)

            # ---- partial products: K^2 | A^2 | K*A  (d on partitions) ----
            sq = sqp.tile([128, 1800], f32)
            nc.scalar.activation(sq[:, 0:600], kat[:, 0:600], AF.Square)
            nc.scalar.activation(sq[:, 600:1200], kat[:, 600:1200], AF.Square)
            nc.vector.tensor_tensor(
                sq[:, 1200:1800], kat[:, 0:600], kat[:, 600:1200], op=OP.mult
            )

            # ---- fold the 6 d-chunks of each quantity: [128,1800]->[128,300]
            tq = foldp.tile([128, 900], f32, tag="tq")
            f300 = foldp.tile([128, 300], f32, tag="f300")
            sq_v = sq[:].rearrange("p (q c j) -> p q c j", q=3, c=6)
            tq_v = tq[:].rearrange("p (q c j) -> p q c j", q=3, c=3)
            f_v = f300[:].rearrange("p (q j) -> p q j", q=3)
            nc.vector.tensor_tensor(tq_v, sq_v[:, :, 0:3], sq_v[:, :, 3:6], op=OP.add)
            nc.vector.tensor_tensor(f_v, tq_v[:, :, 0], tq_v[:, :, 1], op=OP.add)
            nc.vector.tensor_tensor(f_v, f_v, tq_v[:, :, 2], op=OP.add)

            # ---- partition-dim reduce: 3 matmuls into [100,4] columns ----
            red = ps_sp.tile([P_N, 4], f32, tag="sp")
            for q in range(3):
                nc.tensor.matmul(
                    red[:, q : q + 1],
                    f300[:, q * 100 : (q + 1) * 100],
                    ones_col[:],
                    start=True,
                    stop=True,
                )
            sqs = smp.tile([P_N, 3], f32, tag="sqs")
            nc.scalar.copy(sqs[:], red[:, 0:3])

            # ---- rsqrt of ||K||^2, ||A||^2 with one Newton step ----
            srt = smp.tile([P_N, 2], f32, tag="srt")
            nc.scalar.activation(srt[:], sqs[:, 0:2], AF.Sqrt)
            y0 = smp.tile([P_N, 2], f32, tag="y0")
            nc.vector.reciprocal(y0[:], srt[:])
            t1 = smp.tile([P_N, 2], f32, tag="t1")
            nc.vector.tensor_tensor(t1[:], y0[:], y0[:], op=OP.mult)
            nc.vector.tensor_tensor(t1[:], t1[:], sqs[:, 0:2], op=OP.mult)
            nc.vector.tensor_scalar(t1[:], t1[:], -0.5, 1.5, OP.mult, OP.add)
            rs2 = smp.tile([P_N, 2], f32, tag="rs2")
            nc.vector.tensor_tensor(rs2[:], t1[:], y0[:], op=OP.mult)

            # s[p] = <K,A> * rsK * rsA   (column form, for the wt scale)
            s_col = smp.tile([P_N, 1], f32, tag="scol")
            nc.vector.tensor_tensor(s_col[:], rs2[:, 0:1], rs2[:, 1:2], op=OP.mult)
            nc.vector.tensor_tensor(s_col[:], s_col[:], sqs[:, 2:3], op=OP.mult)

            # ---- scores transposed: [100p, 128b] = sum_c K_c.T @ x_c ----
            scT = ps_sp.tile([P_N, BS], f32, tag="sp")
            for c in range(NC6):
                nc.tensor.matmul(
                    scT[:],
                    kat[:, c * P_N : (c + 1) * P_N],
                    xt[:, c * 128 : (c + 1) * 128],
                    start=(c == 0),
                    stop=(c == NC6 - 1),
                )
            su = smp.tile([P_N, BS], f32, tag="su")
            nc.vector.tensor_scalar_mul(su[:], scT[:], rs2[:, 0:1])

            # back to [128b, 100p] for the row top-k
            St = ps_sp.tile([BS, P_N], f32, tag="sp")
            nc.tensor.transpose(St[:], su[:], ident[:P_N, :P_N])
            Ssb = smp.tile([BS, P_N], f32, tag="Ssb")
            nc.vector.tensor_copy(Ssb[:], St[:])

            # ---- top-8 per row in one DVE op; threshold = 5th largest ----
            m8 = smp.tile([BS, 8], f32, tag="m8")
            nc.vector.max(m8[:], Ssb[:])
            mask = smp.tile([BS, P_N], f32, tag="mask")
            nc.vector.tensor_scalar(
                mask[:], Ssb[:], m8[:, TOP_K - 1 : TOP_K], None, OP.is_ge
            )

            # W^T = mask^T * s -> [100, 128]
            mt = ps_sp.tile([P_N, BS], f32, tag="sp")
            nc.tensor.transpose(mt[:], mask[:], ident[:])
            wt = smp.tile([P_N, BS], f32r, tag="wt")
            nc.vector.tensor_scalar_mul(wt[:], mt[:], s_col[:])

            prev = (wt, p_h)

        out_stage(L - 1, *prev)

    nc.compile()
    _CACHE["nc"] = nc
    return nc


def _pack_inputs(x_query, K_all, A_all, P_all):
    x = np.ascontiguousarray(np.asarray(x_query, dtype=np.float32))
    k = np.asarray(K_all, dtype=np.float32)
    a = np.asarray(A_all, dtype=np.float32)
    p = np.ascontiguousarray(
        np.asarray(P_all, dtype=np.float32)
        .reshape(L, P_N, 2, NF // 2)
        .transpose(0, 2, 1, 3)
    )

    def t_pool(m):  # [L,P,D] -> [L,128,6*P]: out[l,p,c*P+j] = m[l,j,c*128+p]
        r = m.transpose(0, 2, 1).reshape(L, NC6, 128, P_N)
        return r.transpose(0, 2, 1, 3).reshape(L, 128, NC6 * P_N)

    kat = np.ascontiguousarray(np.concatenate([t_pool(k), t_pool(a)], axis=2))

    xts = []
    for c in range(N_CORES):
        xc = x[c * BS : (c + 1) * BS]  # [128, L, D]
        # xt[l,p,c6*128+b] = xc[b,l,c6*128+p]
        r = xc.transpose(1, 2, 0).reshape(L, NC6, 128, BS)
        xts.append(
            np.ascontiguousarray(r.transpose(0, 2, 1, 3).reshape(L, BS, D))
        )
    return xts, kat, p


def _run(x_query, K_all, A_all, P_all, trace=False, tmpdir=None):
    from concourse.bass_utils import run_bass_kernel_spmd

    xts, kat, p = _pack_inputs(x_query, K_all, A_all, P_all)

    nc = _build_nc()
    in_maps = [{"xt": xts[c], "kat": kat, "p": p} for c in range(N_CORES)]
    br = run_bass_kernel_spmd(
        nc, in_maps, list(range(N_CORES)), trace=trace, tmpdir=tmpdir
    )
    out = np.stack([r["o"] for r in br.results], axis=0)  # [8, L, BS, NF]
    out = out.transpose(1, 0, 2, 3).reshape(L, B, LP, D)
    return out, br


def kernel(x_query, K_all, A_all, P_all):
    out, _ = _run(x_query, K_all, A_all, P_all)
    return out


# revision 12
# speedup vs baseline: 1.2394x; 1.0158x over previous
"""Trainium2 Bass kernel for nn_L2PppMaskAttn (topk_masking).

Math reformulation of the reference:
  - a_k = sum(l2norm(K[idx]) * l2norm(A[idx])) depends only on (layer, prompt):
    precompute s[l,p] = <K,A> / (||K|| ||A||) once per layer on-device.
  - top-5 ranking over prompts is invariant to q normalization (positive
    per-row scale), so scores u[b,p] = <x[b,l], K[l,p]> / ||K[l,p]|| suffice.
  - out[l,b] = sum_{p in top5} s[l,p] * P[l,p] = (mask_row .* s) @ P_flat[l],
    a dense [B,100] @ [100, 6144] matmul per layer (topk -> masking).

v5 notes (356us baseline -> targeting the ~222us memory roofline):
  - Host passes x, K, A pre-transposed (d on partitions).
  - ||K||^2/||A||^2/<K,A>: square/mult partials, strided 3-op DVE chunk
    fold, then only 3 ones-moving matmuls into a [100,4] column block
    (fp32-stationary matmuls double-emit HI/LO, so keep PE count low).
  - Norm chain entirely in [100,k] column form (single-partition DVE ops
    are ~7x slower; 1-lane only).
  - Scores computed TRANSPOSED so the 1/||K|| scale is a native
    per-partition tensor_scalar; one PE transpose back for the top-k.
  - top-5 threshold via the DVE InstMax top-8 instruction (1 op).
  - DMA triggers must not block compute engines: ALL loads issue from
    nc.sync (pure DMA engine; slot waits harmless), stores from
    nc.scalar (ACT reaches the trigger right after producing ob, so the
    wait is ~0). GpSimd/SWDGE unused.
  - Output matmul pairs share a [128,1024] 2-bank PSUM tile -> 6 wider
    PSUM->SBUF copies per layer instead of 12.

Sharding: data-parallel over batch, 8 cores x 128 rows; K/A/P replicated.
"""

import sys

sys.path.insert(0, "/opt/trn_rl_repo")

import numpy as np

B, L, P_N, LP, D = 1024, 12, 100, 8, 768
N_CORES = 8
BS = B // N_CORES  # 128 batch rows per core
NF = LP * D  # 6144 flattened output features per layer
NC6 = D // 128  # 6 contraction chunks of 128
TOP_K = 5

_CACHE = {}


def _build_nc():
    if "nc" in _CACHE:
        return _CACHE["nc"]

    from contextlib import ExitStack

    import concourse.bass as bass
    import concourse.bacc as bacc
    import concourse.mybir as mybir
    from concourse import masks
    from concourse.tile import TileContext

    f32 = mybir.dt.float32
    f32r = mybir.dt.float32r
    AX = mybir.AxisListType
    OP = mybir.AluOpType
    AF = mybir.ActivationFunctionType

    nc = bacc.Bacc(
        "TRN2",
        target_bir_lowering=False,
        debug=False,
        num_devices=N_CORES,
    )

    # Host-side layouts (see _pack_inputs):
    #  xt[l, p, c*128 + b]        = x_core[b, l, c*128 + p]
    #  kat[l, p, c*100 + j]       = K[l, j, c*128 + p]      (cols 0..599)
    #  kat[l, p, 600 + c*100 + j] = A[l, j, c*128 + p]      (cols 600..1199)
    #  p[l, j, :]                 = P[l, j].reshape(NF)
    xt_d = nc.declare_dram_parameter("xt", [L, BS, D], f32, isOutput=False)
    kat_d = nc.declare_dram_parameter(
        "kat", [L, 128, 2 * NC6 * P_N], f32, isOutput=False
    )
    p_d = nc.declare_dram_parameter("p", [L, 2, P_N, NF // 2], f32r, isOutput=False)
    o_d = nc.declare_dram_parameter("o", [L, BS, NF], f32, isOutput=True)

    with TileContext(nc) as tc, ExitStack() as ctx:
        pool = lambda name, bufs, **kw: ctx.enter_context(
            tc.tile_pool(name=name, bufs=bufs, **kw)
        )
        const = pool("const", 1)
        katp = pool("katp", 6)
        xtp = pool("xtp", 6)
        ppool = pool("pp", 7)
        sqp = pool("sqp", 2)
        foldp = pool("foldp", 2)
        smp = pool("smp", 3)
        obp = pool("obp", 3)
        ps_sp = pool("ps_sp", 4, space="PSUM")  # red/scT/St/mt share one tag
        ps_o = pool("ps_o", 2, space="PSUM")    # [128,1024] = 2 banks each

        ident = const.tile([128, 128], f32)
        masks.make_identity(nc, ident[:])
        ones_col = const.tile([128, 1], f32)
        nc.vector.memset(ones_col[:], 1.0)

        def out_stage(lp, wt, p_h):
            # out[lp] = W @ P_flat: paired-bank PSUM, 2 half stores.
            # Inputs were finished during the previous iteration, so these
            # matmuls/copies fill every engine's dependency stalls while
            # the CURRENT layer's selection chain is still in flight.
            for h in range(2):
                ob = obp.tile([BS, NF // 2], f32)
                for j in range(3):
                    po = ps_o.tile([BS, 1024], f32)
                    for g in range(2):
                        n = j * 2 + g
                        nc.tensor.matmul(
                            po[:, g * 512 : (g + 1) * 512],
                            wt[:],
                            p_h[h][:, n * 512 : (n + 1) * 512],
                            start=True,
                            stop=True,
                        )
                    if j == 0:
                        nc.vector.tensor_copy(
                            ob[:, j * 1024 : (j + 1) * 1024], po[:]
                        )
                    else:
                        nc.scalar.copy(ob[:, j * 1024 : (j + 1) * 1024], po[:])
                nc.gpsimd.dma_start(
                    o_d[lp][:, h * (NF // 2) : (h + 1) * (NF // 2)], ob[:]
                )

        prev = None
        for l in range(L):
            # ---- all loads issue from the SP ring (nc.sync) ----
            kat = katp.tile([128, 2 * NC6 * P_N], f32)
            nc.scalar.dma_start(kat[:], kat_d[l])
            xt = xtp.tile([BS, D], f32)
            nc.scalar.dma_start(xt[:], xt_d[l])
            p_h = []
            for h in range(2):
                ph = ppool.tile([P_N, NF // 2], f32r, tag="ph")
                nc.sync.dma_start(ph[:], p_d[l, h])
                p_h.append(ph)

            # ---- previous layer's output stage (software pipelining) ----
            if prev is not None:
                out_stage(l - 1, *prev)

            # ---- partial products: K^2 | A^2 | K*A  (d on partitions) ----
            sq = sqp.tile([128, 1800], f32)
            nc.scalar.activation(sq[:, 0:600], kat[:, 0:600], AF.Square)
            nc.scalar.activation(sq[:, 600:1200], kat[:, 600:1200], AF.Square)
            nc.vector.tensor_tensor(
                sq[:, 1200:1800], kat[:, 0:600], kat[:, 600:1200], op=OP.mult
            )

            # ---- fold the 6 d-chunks of each quantity: [128,1800]->[128,300]
            tq = foldp.tile([128, 900], f32, tag="tq")
            f300 = foldp.tile([128, 300], f32, tag="f300")
            sq_v = sq[:].rearrange("p (q c j) -> p q c j", q=3, c=6)
            tq_v = tq[:].rearrange("p (q c j) -> p q c j", q=3, c=3)
            f_v = f300[:].rearrange("p (q j) -> p q j", q=3)
            nc.vector.tensor_tensor(tq_v, sq_v[:, :, 0:3], sq_v[:, :, 3:6], op=OP.add)
            nc.vector.tensor_tensor(f_v, tq_v[:, :, 0], tq_v[:, :, 1], op=OP.add)
            nc.vector.tensor_tensor(f_v, f_v, tq_v[:, :, 2], op=OP.add)

            # ---- partition-dim reduce: 3 matmuls into [100,4] columns ----
            red = ps_sp.tile([P_N, 4], f32, tag="sp")
            for q in range(3):
                nc.tensor.matmul(
                    red[:, q : q + 1],
                    f300[:, q * 100 : (q + 1) * 100],
                    ones_col[:],
                    start=True,
                    stop=True,
                )
            sqs = smp.tile([P_N, 3], f32, tag="sqs")
            nc.scalar.copy(sqs[:], red[:, 0:3])

            # ---- rsqrt of ||K||^2, ||A||^2 with one Newton step ----
            srt = smp.tile([P_N, 2], f32, tag="srt")
            nc.scalar.activation(srt[:], sqs[:, 0:2], AF.Sqrt)
            y0 = smp.tile([P_N, 2], f32, tag="y0")
            nc.vector.reciprocal(y0[:], srt[:])
            t1 = smp.tile([P_N, 2], f32, tag="t1")
            nc.vector.tensor_tensor(t1[:], y0[:], y0[:], op=OP.mult)
            nc.vector.tensor_tensor(t1[:], t1[:], sqs[:, 0:2], op=OP.mult)
            nc.vector.tensor_scalar(t1[:], t1[:], -0.5, 1.5, OP.mult, OP.add)
            rs2 = smp.tile([P_N, 2], f32, tag="rs2")
            nc.vector.tensor_tensor(rs2[:], t1[:], y0[:], op=OP.mult)

            # s[p] = <K,A> * rsK * rsA   (column form, for the wt scale)
            s_col = smp.tile([P_N, 1], f32, tag="scol")
            nc.vector.tensor_tensor(s_col[:], rs2[:, 0:1], rs2[:, 1:2], op=OP.mult)
            nc.vector.tensor_tensor(s_col[:], s_col[:], sqs[:, 2:3], op=OP.mult)

            # ---- scores transposed: [100p, 128b] = sum_c K_c.T @ x_c ----
            scT = ps_sp.tile([P_N, BS], f32, tag="sp")
            for c in range(NC6):
                nc.tensor.matmul(
                    scT[:],
                    kat[:, c * P_N : (c + 1) * P_N],
                    xt[:, c * 128 : (c + 1) * 128],
                    start=(c == 0),
                    stop=(c == NC6 - 1),
                )
            su = smp.tile([P_N, BS], f32, tag="su")
            nc.vector.tensor_scalar_mul(su[:], scT[:], rs2[:, 0:1])

            # back to [128b, 100p] for the row top-k
            St = ps_sp.tile([BS, P_N], f32, tag="sp")
            nc.tensor.transpose(St[:], su[:], ident[:P_N, :P_N])
            Ssb = smp.tile([BS, P_N], f32, tag="Ssb")
            nc.vector.tensor_copy(Ssb[:], St[:])

            # ---- top-8 per row in one DVE op; threshold = 5th largest ----
            m8 = smp.tile([BS, 8], f32, tag="m8")
            nc.vector.max(m8[:], Ssb[:])
            mask = smp.tile([BS, P_N], f32, tag="mask")
            nc.vector.tensor_scalar(
                mask[:], Ssb[:], m8[:, TOP_K - 1 : TOP_K], None, OP.is_ge
            )

            # W^T = mask^T * s -> [100, 128]
            mt = ps_sp.tile([P_N, BS], f32, tag="sp")
            nc.tensor.transpose(mt[:], mask[:], ident[:])
            wt = smp.tile([P_N, BS], f32r, tag="wt")
            nc.vector.tensor_scalar_mul(wt[:], mt[:], s_col[:])

            prev = (wt, p_h)

        out_stage(L - 1, *prev)

    nc.compile()
    _CACHE["nc"] = nc
    return nc


def _pack_inputs(x_query, K_all, A_all, P_all):
    x = np.ascontiguousarray(np.asarray(x_query, dtype=np.float32))
    k = np.asarray(K_all, dtype=np.float32)
    a = np.asarray(A_all, dtype=np.float32)
    p = np.ascontiguousarray(
        np.asarray(P_all, dtype=np.float32)
        .reshape(L, P_N, 2, NF // 2)
        .transpose(0, 2, 1, 3)
    )

    def t_pool(m):  # [L,P,D] -> [L,128,6*P]: out[l,p,c*P+j] = m[l,j,c*128+p]
        r = m.transpose(0, 2, 1).reshape(L, NC6, 128, P_N)
        return r.transpose(0, 2, 1, 3).reshape(L, 128, NC6 * P_N)

    kat = np.ascontiguousarray(np.concatenate([t_pool(k), t_pool(a)], axis=2))

    xts = []
    for c in range(N_CORES):
        xc = x[c * BS : (c + 1) * BS]  # [128, L, D]
        # xt[l,p,c6*128+b] = xc[b,l,c6*128+p]
        r = xc.transpose(1, 2, 0).reshape(L, NC6, 128, BS)
        xts.append(
            np.ascontiguousarray(r.transpose(0, 2, 1, 3).reshape(L, BS, D))
        )
    return xts, kat, p


def _run(x_query, K_all, A_all, P_all, trace=False, tmpdir=None):
    from concourse.bass_utils import run_bass_kernel_spmd

    xts, kat, p = _pack_inputs(x_query, K_all, A_all, P_all)

    nc = _build_nc()
    in_maps = [{"xt": xts[c], "kat": kat, "p": p} for c in range(N_CORES)]
    br = run_bass_kernel_spmd(
        nc, in_maps, list(range(N_CORES)), trace=trace, tmpdir=tmpdir
    )
    out = np.stack([r["o"] for r in br.results], axis=0)  # [8, L, BS, NF]
    out = out.transpose(1, 0, 2, 3).reshape(L, B, LP, D)
    return out, br


def kernel(x_query, K_all, A_all, P_all):
    out, _ = _run(x_query, K_all, A_all, P_all)
    return out
